# revision 28
# baseline (speedup 1.0000x reference)
"""Trainium2 Bass kernel for nn_Attention (B=4, S=2048, D=1024, DK=256).

Computation (reference, per batch b):
    qp = q @ Wq.T            [S, DK]
    kp = q @ Wk.T            [S, DK]
    scores = qp @ kp.T / sqrt(DK)
    attn = softmax(scores, axis=-1)
    out = attn @ q           (v = q)
    y = out @ Wv.T           [S, D]

Sharding: 8 cores = 4 batches x 2 query-halves. Each core handles one batch's
full key/value sequence and one 1024-row query half. The host "rolls" the
sequence per core so that the core's query half occupies rows 0..1023; since
softmax is invariant to key permutation this changes nothing numerically.

Per-core dataflow (all matmul operands float32r: ~1.5e-4 rms error, 1 cyc/row
at N>=256). Everything is software-pipelined against the DMA stream: qt
arrives in 512-column chunks, the projections consume each chunk as it lands,
and chunk-0 score matmuls trail the kp chunks immediately.

    inputs: qn [S, D] (rolled, natural), qT [D, S] (rolled, transposed),
            wqT/wkT [D, DK], wvT [D, D]   (weights pre-transposed on host)
    kpT[e, s_k] = wkT.T @ qT              (per 512-col chunk, acc over d)
    qpT[e, s_q] = wqT.T @ qT[:, :1024]
    per s_q chunk of 512:
      scoresT[s_k, s_q] = kpT.T @ qpT     (16 k-tiles x 2 e-acc)
      expT = exp(scoresT / 16)            (ScalarE, PSUM->SBUF, fused scale)
      denom: DVE-accumulate expT over k -> ones-matmul partition sum ->
             PE-transpose 128-blocks -> reciprocal -> recip[s_q part, 1]
      unnormT[d, s_q] = qn.T @ expT       (8 d-tiles x 16 k-acc, 2 groups of 4)
      y[s_q, e_out] = unnormT.T @ wvT     (8 d-acc)
      y *= recip (per-partition) -> DMA out

PSUM discipline (8 banks): tag "acc" bufs=4 (qp accumulators, then unnorm
groups), tag "sc" bufs=3 (kp accumulators, score tiles, y tiles), "pd" 1.
"""

import numpy as np

import concourse.mybir as mybir
import concourse.tile as tile
from concourse import bacc
from concourse.bass_utils import run_bass_kernel_spmd
from concourse.masks import make_identity

B, S, D, DK = 4, 2048, 1024, 256
SQ = S // 2  # query rows per core
P = 128
CH = 512  # s_q chunk width
NC = S // 512  # 4 qt column chunks
N_CORES = 8

FR = mybir.dt.float32r
F32 = mybir.dt.float32

KT = S // P  # 16 key tiles
DT = D // P  # 8 d tiles
ET = DK // P  # 2 e tiles

_PROGRAM = None


def _build_program():
    nc = bacc.Bacc(None, target_bir_lowering=False, debug=False)

    qn_d = nc.dram_tensor("qn", [S, D], FR, kind="ExternalInput")
    # Matmul operands live in narrow dedicated SBUF tiles (small partition
    # stride = fast LDWEIGHTS/streaming); their DRAM sources are host-packed
    # so each tile's DMA reads a fully contiguous block.
    qt_d = nc.dram_tensor("qt", [NC * DT * P, 512], FR, kind="ExternalInput")
    wqt_d = nc.dram_tensor("wqt", [D, DK], FR, kind="ExternalInput")
    wkt_d = nc.dram_tensor("wkt", [D, DK], FR, kind="ExternalInput")
    wvt_d = nc.dram_tensor("wvt", [DT * 2 * P, 512], FR, kind="ExternalInput")
    y_d = nc.dram_tensor("y", [SQ, D], F32, kind="ExternalOutput")

    with tile.TileContext(nc) as tc:
        with (
            tc.tile_pool(name="pp", bufs=1) as pp,
            tc.tile_pool(name="ps", bufs=1, space="PSUM") as ps,
        ):
            # ---- constants + warmup ----
            ones_f = pp.tile([P, 1], F32, tag="ones_f")
            nc.vector.memset(ones_f[:], 1.0)
            ones = pp.tile([P, 1], FR, tag="ones")
            nc.vector.tensor_copy(ones[:], ones_f[:])
            ident = pp.tile([P, P], F32, tag="ident")
            make_identity(nc, ident[:])
            # Warm the ACT exp table-set (~2.7us first-call cost) early.
            warm_act = pp.tile([P, 1], F32, tag="warm_act")
            nc.scalar.activation(
                warm_act[:], ones_f[:], mybir.ActivationFunctionType.Exp
            )
            # HAM warmup + boot->first-data bridge: dummy matmuls with no data
            # deps. Results are never read.
            warm_f = pp.tile([P, 512], F32, tag="warm_f")
            nc.vector.memset(warm_f[:], 1.0)
            warm_r = pp.tile([P, 512], FR, tag="warm_r")
            nc.vector.tensor_copy(warm_r[:], warm_f[:])
            pwarm = ps.tile([P, 512], F32, tag="sc", bufs=3, name="pwarm")
            for _ in range(36):
                nc.tensor.matmul(
                    pwarm[:], warm_r[:, :P], warm_r[:], start=True, stop=True
                )

            # ---- persistent arrays ----
            qn = [pp.tile([P, D], FR, tag="qn", bufs=KT, name=f"qn{k}") for k in range(KT)]
            kpt = {
                (e, n): pp.tile([P, 512], FR, tag="kpt", bufs=ET * NC, name=f"kpt{e}_{n}")
                for e in range(ET)
                for n in range(NC)
            }
            qpt = {
                (e, c): pp.tile([P, CH], FR, tag="qpt", bufs=ET * 2, name=f"qpt{e}_{c}")
                for e in range(ET)
                for c in range(2)
            }
            expt = {}  # (chunk, k) -> tile, allocated on the fly (tag-rotated)

            # DMA stream: all on the SP HWDGE queue in priority order
            # (weights -> qt chunks -> qn -> wvt). qt/wq/wk live in a nested
            # pool released after the projections; wvt/unsb/ysb reuse that
            # space afterwards.

            # ---- helpers ----
            def scores_block(c, ks):
                """scoresT + exp for key tiles ks of chunk c."""
                for k in ks:
                    sc = ps.tile([P, CH], F32, tag="sc", bufs=3, name=f"sc{c}_{k}")
                    for e in range(ET):
                        nc.tensor.matmul(
                            sc[:],
                            kpt[e, k // 4][:, (k % 4) * P : (k % 4 + 1) * P],
                            qpt[e, c][:],
                            start=(e == 0),
                            stop=(e == ET - 1),
                        )
                    ex = pp.tile([P, CH], FR, tag="expt", bufs=16, name=f"ex{c}_{k}")
                    nc.scalar.activation(
                        ex[:], sc[:], mybir.ActivationFunctionType.Exp, scale=1.0 / 16.0
                    )
                    expt[c, k] = ex

            def proj_chunk(n, with_qp):
                """kp (and qp if with_qp) for qt column chunk n, acc over d."""
                pks = {
                    e: ps.tile([P, 512], F32, tag="sc", bufs=3, name=f"pk{e}_{n}")
                    for e in range(ET)
                }
                pqs = (
                    {
                        e: ps.tile([P, 512], F32, tag="acc", bufs=4, name=f"pq{e}_{n}")
                        for e in range(ET)
                    }
                    if with_qp
                    else {}
                )
                for d in range(DT):
                    rhs = qt[n, d][:]
                    for e in range(ET):
                        nc.tensor.matmul(
                            pks[e][:],
                            wkt[d][:, e * P : (e + 1) * P],
                            rhs,
                            start=(d == 0),
                            stop=(d == DT - 1),
                        )
                        if with_qp:
                            nc.tensor.matmul(
                                pqs[e][:],
                                wqt[d][:, e * P : (e + 1) * P],
                                rhs,
                                start=(d == 0),
                                stop=(d == DT - 1),
                            )
                for e in range(ET):
                    nc.vector.tensor_copy(kpt[e, n][:], pks[e][:])
                    if with_qp:
                        nc.vector.tensor_copy(qpt[e, n][:], pqs[e][:])

            def denom_block(c):
                dacc = pp.tile([P, CH], F32, tag="dacc", bufs=1, name=f"dacc{c}")
                nc.vector.tensor_copy(dacc[:], expt[c, 0][:])
                for k in range(1, KT):
                    nc.vector.tensor_tensor(
                        dacc[:], dacc[:], expt[c, k][:], op=mybir.AluOpType.add
                    )
                daccr = pp.tile([P, CH], FR, tag="daccr", bufs=1, name=f"daccr{c}")
                nc.vector.tensor_copy(daccr[:], dacc[:])
                pd = ps.tile([1, CH], F32, tag="pd", bufs=1, name=f"pd{c}")
                nc.tensor.matmul(pd[:], ones[:], daccr[:], start=True, stop=True)
                drow = pp.tile([1, CH], F32, tag="drow", bufs=2, name=f"drow{c}")
                nc.vector.tensor_copy(drow[:], pd[:])
                pt = ps.tile([P, CH // P], F32, tag="pd", bufs=1, name=f"pt{c}")
                for j in range(CH // P):
                    nc.tensor.transpose(
                        pt[:, j : j + 1], drow[:1, j * P : (j + 1) * P], ident[:1, :1]
                    )
                recip = pp.tile([P, CH // P], F32, tag="recip", bufs=2, name=f"recip{c}")
                nc.vector.reciprocal(recip[:], pt[:])
                return recip

            def unnorm_block(c, p2):
                unsb = []
                for g in range(2):
                    accs = [
                        ps.tile([P, CH], F32, tag="acc", bufs=4, name=f"un{c}_{g}_{i}")
                        for i in range(4)
                    ]
                    for k in range(KT):
                        for i in range(4):
                            d = g * 4 + i
                            nc.tensor.matmul(
                                accs[i][:],
                                qn[k][:, d * P : (d + 1) * P],
                                expt[c, k][:],
                                start=(k == 0),
                                stop=(k == KT - 1),
                            )
                    for i in range(4):
                        us = p2.tile([P, CH], FR, tag="unsb", bufs=8, name=f"us{c}_{g}_{i}")
                        nc.vector.tensor_copy(us[:], accs[i][:])
                        unsb.append(us)
                return unsb

            def y_block(c, unsb, recip, wvn, p2):
                cs = c * CH
                for m in range(CH // P):
                    for n in range(D // 512):
                        yb = ps.tile([P, 512], F32, tag="sc", bufs=3, name=f"yb{c}_{m}_{n}")
                        for d in range(DT):
                            nc.tensor.matmul(
                                yb[:],
                                unsb[d][:, m * P : (m + 1) * P],
                                wvn[d, n][:],
                                start=(d == 0),
                                stop=(d == DT - 1),
                            )
                        ys = p2.tile([P, 512], F32, tag="ysb", bufs=4, name=f"ys{c}_{m}_{n}")
                        nc.vector.tensor_scalar_mul(ys[:], yb[:], recip[:, m : m + 1])
                        nc.sync.dma_start(
                            y_d[cs + m * P : cs + (m + 1) * P, n * 512 : (n + 1) * 512],
                            ys[:],
                        )

            # ---- schedule (trace order == PE priority order) ----
            # Phase-1 pool: qt chunks (rolling window of 12 tiles) + wq/wk.
            # Projections stream against arriving qt chunks; chunk-0 score
            # matmuls trail each kp chunk so PE never waits on the full qt.
            with tc.tile_pool(name="ph1", bufs=1) as p1:
                wkt = [
                    p1.tile([P, DK], FR, tag="wkt", bufs=DT, name=f"wkt{d}")
                    for d in range(DT)
                ]
                wqt = [
                    p1.tile([P, DK], FR, tag="wqt", bufs=DT, name=f"wqt{d}")
                    for d in range(DT)
                ]
                for d in range(DT):
                    nc.sync.dma_start(wkt[d][:], wkt_d[d * P : (d + 1) * P, :])
                    nc.sync.dma_start(wqt[d][:], wqt_d[d * P : (d + 1) * P, :])
                qt = {}
                for n in range(NC):
                    for d in range(DT):
                        t = p1.tile([P, 512], FR, tag="qt", bufs=12, name=f"qt{n}_{d}")
                        blk = (n * DT + d) * P
                        nc.sync.dma_start(t[:], qt_d[blk : blk + P, :])
                        qt[n, d] = t
                for k in range(KT):
                    nc.sync.dma_start(qn[k][:], qn_d[k * P : (k + 1) * P, :])

                proj_chunk(0, with_qp=True)
                scores_block(0, range(0, 4))
                proj_chunk(1, with_qp=True)
                scores_block(0, range(4, 8))
                proj_chunk(2, with_qp=False)
                scores_block(0, range(8, 12))
                proj_chunk(3, with_qp=False)

            # Phase-2 pool reuses the qt/wq/wk space for wvt, unnorm and y
            # staging.
            with tc.tile_pool(name="ph2", bufs=1) as p2:
                wvn = {}
                for dd in range(DT):
                    for n in range(2):
                        t = p2.tile([P, 512], FR, tag="wvt", bufs=DT * 2, name=f"wv{dd}_{n}")
                        blk = (dd * 2 + n) * P
                        nc.sync.dma_start(t[:], wvt_d[blk : blk + P, :])
                        wvn[dd, n] = t

                scores_block(0, range(12, 16))
                recip0 = denom_block(0)
                unsb0 = unnorm_block(0, p2)
                # chunk-1 scores fill the PE while wvt / qn tails stream in
                scores_block(1, range(0, 16))
                recip1 = denom_block(1)
                y_block(0, unsb0, recip0, wvn, p2)
                unsb1 = unnorm_block(1, p2)
                y_block(1, unsb1, recip1, wvn, p2)

    nc.compile()
    return nc


def build_in_maps(q, Wq, Wk, Wv):
    q = np.ascontiguousarray(np.asarray(q, dtype=np.float32))

    wqt = np.ascontiguousarray(np.asarray(Wq, dtype=np.float32).T)
    wkt = np.ascontiguousarray(np.asarray(Wk, dtype=np.float32).T)
    # wv blocks: (d, n) -> [128, 512] with row p = wvT[d*128+p, n*512:(n+1)*512]
    wvt = np.ascontiguousarray(
        np.asarray(Wv, dtype=np.float32)
        .T.reshape(DT, P, 2, 512)
        .transpose(0, 2, 1, 3)
        .reshape(DT * 2 * P, 512)
    )

    in_maps = []
    for core in range(N_CORES):
        b, h = divmod(core, 2)
        qb = q[b]
        rolled = np.concatenate([qb[h * SQ : (h + 1) * SQ], qb[(1 - h) * SQ : (2 - h) * SQ]])
        qT = rolled.T  # [D, S]
        # qt blocks: (n, d) -> [128, 512] with row p = qT[d*128+p, n*512:(n+1)*512]
        qt_packed = np.ascontiguousarray(
            qT.reshape(DT, P, NC, 512).transpose(2, 0, 1, 3).reshape(NC * DT * P, 512)
        )
        in_maps.append(
            {
                "qn": np.ascontiguousarray(rolled),
                "qt": qt_packed,
                "wqt": wqt,
                "wkt": wkt,
                "wvt": wvt,
            }
        )
    return in_maps


def kernel(q, Wq, Wk, Wv):
    global _PROGRAM
    if _PROGRAM is None:
        _PROGRAM = _build_program()
    nc = _PROGRAM
    in_maps = build_in_maps(q, Wq, Wk, Wv)
    res = run_bass_kernel_spmd(nc, in_maps, list(range(N_CORES)))

    out = np.empty((B, S, D), dtype=np.float32)
    for core in range(N_CORES):
        b, h = divmod(core, 2)
        out[b, h * SQ : (h + 1) * SQ, :] = res.results[core]["y"]
    return out
